# revision 11
# baseline (speedup 1.0000x reference)
"""Trainium2 Bass kernel: masked-bilinear channel-mixing Conv2d.

reference math (N=4, C=96, H=W=32, O=96, K=3, PAD=1):
    p = avgpool3x3(x, count_include_pad) -> [N, C, H, W] -> [N, L=1024, C]
    wm = weight * mask                              [O, C, C]
    y[n,l,o] = sum_{c,d} wm[o,c,d] p[n,l,c] p[n,l,d] + bias[o]

Sharding: data-parallel over the 4096 spatial locations -> 8 cores, each
takes half of one image (16 rows = 512 locations) and computes all 96
output channels. Weight/mask are replicated (host pre-transposes to
c-major, block-interleaved, so each block arrives in one contiguous DMA);
the avg-pool 1/9 scale and the weight*mask product run on device.

Per-core device pipeline (bf16 matmul operands, f32 PSUM accumulate):
  warmup: dummy matmul burst during the DMA/pool phase (the PE clock gate
          takes ~16us of sustained activity to lift 1.2GHz -> 2.4GHz)
  pooling (4 DVE adds + scale)        -> pt16 [96(c), 512(loc)] bf16
  wm16 = wt * mt                      -> [96(c), 9216(o,d)] bf16 (GPSIMD)
  o-loop, triples (3m, 3m+1, 3m+2):
    T_o  = matmul(lhsT=wm16[:, o], rhs=pt16)     # [96(d), 512] f32 PSUM
    z    = (T ⊙ pt16) batched per triple, routed to one of:
             direct DVE TT (PSUM src) | ACT copy + DVE bf16 2x TT |
             ACT copy + GPSIMD TT
    y[r(o)] = matmul(lhsT=onehot32, rhs=z_o)     # M=32; r(o)=32*(o%3)+o//3
             consecutive o's hit PE column-groups 0/1/2 -> 3x concurrent,
             and a triple shares one lhsT (same one-hot column m=o//3)
  y_sb = y + bias_perm; DMA out un-permutes rows via a reordered DRAM AP.
"""
import numpy as np

import concourse.bass as bass
import concourse.bacc as bacc
import concourse.mybir as mybir
from concourse import tile
from concourse import bass_utils

C = 96
O = 96
HS = 16           # rows per core shard
W = 32
L = HS * W        # 512 locations per core
N_CORES = 8
NBLK = 8          # weight/mask DMA + multiply blocks
WARMUP_MMS = 10
KEEPWARM_FROM = 5      # triples >= this get keep-warm dummy matmuls
KEEPWARM_PER_TRIPLE = 2
F32 = mybir.dt.float32
BF16 = mybir.dt.bfloat16

# z-route per o-triple (m = o//3): R3 via ACT copy + GPSIMD TT,
# R1 direct DVE TT from PSUM, R2 (default) ACT copy + DVE bf16 2x TT.
R3_TRIPLES = {5, 10, 15, 20, 25, 30}
R1_TRIPLES = {2, 8, 14, 22, 28}
WM_DVE_BLOCKS = {0, 2, 4, 6}   # rest go to GPSIMD


def _build_kernel(nc: bass.Bass):
    xs_d = nc.dram_tensor("xs", [C, 18 * 34], F32, kind="ExternalInput")
    wmcat_d = nc.dram_tensor("wmcat", [C, 2 * O * C], F32, kind="ExternalInput")
    b_d = nc.dram_tensor("bias", [O, 1], F32, kind="ExternalInput")
    y_d = nc.dram_tensor("y", [O, L], F32, kind="ExternalOutput")

    with tile.TileContext(nc) as tc:
        with (
            tc.tile_pool(name="const", bufs=1) as cpool,
            tc.tile_pool(name="work", bufs=1) as wpool,
            tc.tile_pool(name="tc3", bufs=3) as tcpool,
            tc.tile_pool(name="z", bufs=3) as zpool,
            tc.tile_pool(name="tpsum", bufs=2, space="PSUM") as tpsum,
            tc.tile_pool(name="ypsum", bufs=1, space="PSUM") as ypsum,
            tc.tile_pool(name="wpsum", bufs=1, space="PSUM") as wpsum,
        ):
            xs = cpool.tile([C, 18 * 34], F32)
            wmcat = cpool.tile([C, 2 * O * C], F32)
            wm16 = cpool.tile([C, O * C], BF16)
            bias = cpool.tile([O, 1], F32)
            # zob[:, 31] is ones, else zero; zob[:, 31-m : 63-m] is [96, 32]
            # with ones in column m -> as lhsT it scatters the partition-sum
            # of rhs into row (32*colgroup + m) of the output.
            zob = cpool.tile([C, 63], BF16)
            warm16 = cpool.tile([C, L], BF16)
            nc.sync.dma_start(xs[:], xs_d.ap())
            nc.sync.dma_start(bias[:], b_d.ap())
            nc.vector.memset(zob[:], 0.0)
            nc.vector.memset(zob[:, 31:32], 1.0)
            nc.vector.memset(warm16[:], 0.0)

            # PE warmup: garbage matmuls while DMA/pool/wm stages run
            wps = wpsum.tile([C, L], F32)
            for _ in range(WARMUP_MMS):
                nc.tensor.matmul(wps[:], warm16[:, 0:C], warm16[:],
                                 start=True, stop=True, skip_group_check=True)

            # weight/mask: host packs [blk] = [wt_blk | mt_blk] so each wm
            # block waits on exactly one DMA-queue semaphore; multiplies
            # overlap the transfers.
            BL = O * C // NBLK
            for blk in range(NBLK):
                base = blk * 2 * BL
                nc.sync.dma_start(wmcat[:, base:base + 2 * BL],
                                  wmcat_d.ap()[:, base:base + 2 * BL])
                wm_eng = nc.vector if blk in WM_DVE_BLOCKS else nc.gpsimd
                wm_eng.tensor_mul(
                    wm16[:, blk * BL:(blk + 1) * BL],
                    wmcat[:, base:base + BL],
                    wmcat[:, base + BL:base + 2 * BL])

            # --- pooling: horizontal then vertical 3-tap box sums ---
            s1 = wpool.tile([C, 18 * 32], F32)
            s2 = wpool.tile([C, 18 * 32], F32)
            pt_raw = wpool.tile([C, L], F32)
            pt16 = wpool.tile([C, L], BF16)
            pt2 = wpool.tile([C, L], F32)
            x3 = xs[:].rearrange("c (h w) -> c h w", h=18)
            s1v = s1[:].rearrange("c (h w) -> c h w", h=18)
            s2v = s2[:].rearrange("c (h w) -> c h w", h=18)
            nc.vector.tensor_add(s1v, x3[:, :, 0:32], x3[:, :, 1:33])
            nc.vector.tensor_add(s2v, s1v, x3[:, :, 2:34])
            ptv = pt_raw[:].rearrange("c (h w) -> c h w", h=HS)
            pt2v = pt2[:].rearrange("c (h w) -> c h w", h=HS)
            nc.vector.tensor_add(pt2v, s2v[:, 0:16, :], s2v[:, 1:17, :])
            nc.vector.tensor_add(ptv, pt2v, s2v[:, 2:18, :])
            # p = boxsum/9; p enters the quadratic form twice -> 1/81 total
            nc.vector.tensor_scalar_mul(pt16[:], pt_raw[:], 1.0 / 9.0)

            y_ps = ypsum.tile([O, L], F32)
            for m in range(32):
                os3 = (3 * m, 3 * m + 1, 3 * m + 2)
                t3 = tpsum.tile([C, 3 * L], F32)
                for k, o in enumerate(os3):
                    nc.tensor.matmul(
                        t3[:, k * L:(k + 1) * L],
                        wm16[:, o * C:(o + 1) * C], pt16[:],
                        start=True, stop=True,
                    )
                z3 = zpool.tile([C, 3 * L], BF16, tag="z3")
                z3v = z3[:].rearrange("c (k l) -> c k l", k=3)
                pt3 = pt16[:].unsqueeze(1).broadcast_to((C, 3, L))
                if m in R1_TRIPLES:
                    nc.vector.tensor_mul(
                        z3v, t3[:].rearrange("c (k l) -> c k l", k=3), pt3)
                elif m in R3_TRIPLES:
                    tc3f = tcpool.tile([C, 3 * L], F32, tag="tc3f")
                    nc.scalar.activation(
                        tc3f[:], t3[:], mybir.ActivationFunctionType.Copy)
                    nc.gpsimd.tensor_mul(
                        z3v, tc3f[:].rearrange("c (k l) -> c k l", k=3), pt3)
                else:
                    tc3 = tcpool.tile([C, 3 * L], BF16, tag="tc3")
                    nc.scalar.activation(
                        tc3[:], t3[:], mybir.ActivationFunctionType.Copy)
                    nc.vector.tensor_mul(
                        z3v, tc3[:].rearrange("c (k l) -> c k l", k=3), pt3)
                for k, o in enumerate(os3):
                    # y row r(o) = 32*(o%3) + o//3 -> col group k, column m
                    nc.tensor.matmul(
                        y_ps[32 * k:32 * (k + 1), :],
                        zob[:, 31 - m:63 - m], z3[:, k * L:(k + 1) * L],
                        start=(m == 0), stop=(m == 31),
                        skip_group_check=True,
                    )
                if m >= KEEPWARM_FROM:
                    # dummy matmuls keep the PE activity monitor at full
                    # clock while the z-pipeline engines catch up
                    for _ in range(KEEPWARM_PER_TRIPLE):
                        nc.tensor.matmul(
                            wps[:], warm16[:, 0:C], warm16[:],
                            start=True, stop=True, skip_group_check=True)

            y_sb = wpool.tile([O, L], F32)
            nc.vector.tensor_scalar_add(y_sb[:], y_ps[:], bias[:])
            # un-permute rows: SBUF row 32j+m holds output channel o=3m+j
            y_out = y_d.ap().rearrange("(m j) l -> j m l", j=3)
            nc.sync.dma_start(y_out, y_sb[:])
    return nc


_NC_CACHE = {}


def _get_nc():
    if "nc" not in _NC_CACHE:
        nc = bacc.Bacc("TRN2", target_bir_lowering=False, debug=False,
                       enable_asserts=False)
        _build_kernel(nc)
        nc.compile()
        _NC_CACHE["nc"] = nc
    return _NC_CACHE["nc"]


def _prep_shards(x, weight, mask, bias):
    xpad = np.pad(np.asarray(x, np.float32), ((0, 0), (0, 0), (1, 1), (1, 1)))
    wt = np.asarray(weight, np.float32).transpose(1, 0, 2).reshape(C, O * C)
    mt = np.asarray(mask, np.float32).transpose(1, 0, 2).reshape(C, O * C)
    BL = O * C // NBLK
    wmcat = np.empty((C, NBLK, 2, BL), np.float32)
    wmcat[:, :, 0, :] = wt.reshape(C, NBLK, BL)
    wmcat[:, :, 1, :] = mt.reshape(C, NBLK, BL)
    wmcat = np.ascontiguousarray(wmcat.reshape(C, 2 * O * C))
    # device keeps y rows permuted (row 32j+m = channel 3m+j); bias must
    # be laid out the same way
    b = np.asarray(bias, np.float32).reshape(32, 3).T.reshape(O, 1)
    b = np.ascontiguousarray(b)
    in_maps = []
    for core in range(N_CORES):
        n, half = core // 2, core % 2
        h0 = half * HS
        xs = np.ascontiguousarray(
            xpad[n, :, h0:h0 + 18, :].reshape(C, 18 * 34))
        in_maps.append({"xs": xs, "wmcat": wmcat, "bias": b})
    return in_maps


def run_sharded(x, weight, mask, bias, **run_kwargs):
    """Run on the 8 NeuronCores; returns (y_full, BassKernelResults)."""
    nc = _get_nc()
    in_maps = _prep_shards(x, weight, mask, bias)
    res = bass_utils.run_bass_kernel_spmd(
        nc, in_maps, core_ids=list(range(N_CORES)), **run_kwargs)
    n_img = np.asarray(x).shape[0]
    y = np.empty((n_img, O, 32, 32), dtype=np.float32)
    for core in range(N_CORES):
        n, half = core // 2, core % 2
        h0 = half * HS
        y[n, :, h0:h0 + HS, :] = res.results[core]["y"].reshape(O, HS, W)
    return y, res


def kernel(x, weight, mask, bias):
    y, _ = run_sharded(x, weight, mask, bias)
    return y


# revision 12
# speedup vs baseline: 1.0530x; 1.0530x over previous
"""Trainium2 Bass kernel: masked-bilinear channel-mixing Conv2d.

reference math (N=4, C=96, H=W=32, O=96, K=3, PAD=1):
    p = avgpool3x3(x, count_include_pad) -> [N, C, H, W] -> [N, L=1024, C]
    wm = weight * mask                              [O, C, C]
    y[n,l,o] = sum_{c,d} wm[o,c,d] p[n,l,c] p[n,l,d] + bias[o]

Sharding: data-parallel over the 4096 spatial locations -> 8 cores, each
takes half of one image (16 rows = 512 locations) and computes all 96
output channels. Weight/mask are replicated (host pre-transposes to
c-major, block-interleaved, so each block arrives in one contiguous DMA);
the avg-pool 1/9 scale and the weight*mask product run on device.

Per-core device pipeline (bf16 matmul operands, f32 PSUM accumulate):
  warmup: dummy matmul burst during the DMA/pool phase (the PE clock gate
          takes ~16us of sustained activity to lift 1.2GHz -> 2.4GHz)
  pooling (4 DVE adds + scale)        -> pt16 [96(c), 512(loc)] bf16
  wm16 = wt * mt                      -> [96(c), 9216(o,d)] bf16 (GPSIMD)
  o-loop, triples (3m, 3m+1, 3m+2):
    T_o  = matmul(lhsT=wm16[:, o], rhs=pt16)     # [96(d), 512] f32 PSUM
    z    = (T ⊙ pt16) batched per triple, routed to one of:
             direct DVE TT (PSUM src) | ACT copy + DVE bf16 2x TT |
             ACT copy + GPSIMD TT
    y[r(o)] = matmul(lhsT=onehot32, rhs=z_o)     # M=32; r(o)=32*(o%3)+o//3
             consecutive o's hit PE column-groups 0/1/2 -> 3x concurrent,
             and a triple shares one lhsT (same one-hot column m=o//3)
  y_sb = y + bias_perm; DMA out un-permutes rows via a reordered DRAM AP.
"""
import numpy as np

import concourse.bass as bass
import concourse.bacc as bacc
import concourse.mybir as mybir
from concourse import tile
from concourse import bass_utils

C = 96
O = 96
HS = 16           # rows per core shard
W = 32
L = HS * W        # 512 locations per core
N_CORES = 8
NBLK = 8          # weight/mask DMA + multiply blocks
WARMUP_MMS = 12
KEEPWARM_FROM = 99     # disabled
KEEPWARM_PER_TRIPLE = 0
F32 = mybir.dt.float32
BF16 = mybir.dt.bfloat16

# z-route per o-triple (m = o//3): R3 via ACT copy + GPSIMD TT,
# R1 direct DVE TT from PSUM, R2 (default) ACT copy + DVE bf16 2x TT.
R3_TRIPLES = set()
R1_TRIPLES = {3, 7, 11, 15, 19, 23, 27, 31}
WM_DVE_BLOCKS = {0, 1}   # rest go to GPSIMD


def _build_kernel(nc: bass.Bass):
    xs_d = nc.dram_tensor("xs", [C, 18 * 34], F32, kind="ExternalInput")
    wmcat_d = nc.dram_tensor("wmcat", [C, 2 * O * C], F32, kind="ExternalInput")
    b_d = nc.dram_tensor("bias", [O, 1], F32, kind="ExternalInput")
    y_d = nc.dram_tensor("y", [O, L], F32, kind="ExternalOutput")

    with tile.TileContext(nc) as tc:
        with (
            tc.tile_pool(name="const", bufs=1) as cpool,
            tc.tile_pool(name="work", bufs=1) as wpool,
            tc.tile_pool(name="tc3", bufs=3) as tcpool,
            tc.tile_pool(name="z", bufs=3) as zpool,
            tc.tile_pool(name="tpsum", bufs=2, space="PSUM") as tpsum,
            tc.tile_pool(name="ypsum", bufs=1, space="PSUM") as ypsum,
            tc.tile_pool(name="wpsum", bufs=1, space="PSUM") as wpsum,
        ):
            xs = cpool.tile([C, 18 * 34], F32)
            wmcat = cpool.tile([C, 2 * O * C], F32)
            wm16 = cpool.tile([C, O * C], BF16)
            bias = cpool.tile([O, 1], F32)
            # zob[:, 31] is ones, else zero; zob[:, 31-m : 63-m] is [96, 32]
            # with ones in column m -> as lhsT it scatters the partition-sum
            # of rhs into row (32*colgroup + m) of the output.
            zob = cpool.tile([C, 63], BF16)
            warm16 = cpool.tile([C, L], BF16)
            nc.sync.dma_start(xs[:], xs_d.ap())
            nc.sync.dma_start(bias[:], b_d.ap())
            nc.vector.memset(zob[:], 0.0)
            nc.vector.memset(zob[:, 31:32], 1.0)
            nc.vector.memset(warm16[:], 0.0)

            # PE warmup: garbage matmuls while DMA/pool/wm stages run
            wps = wpsum.tile([C, L], F32)
            for _ in range(WARMUP_MMS):
                nc.tensor.matmul(wps[:], warm16[:, 0:C], warm16[:],
                                 start=True, stop=True, skip_group_check=True)

            # weight/mask: host packs [blk] = [wt_blk | mt_blk] so each wm
            # block waits on exactly one DMA-queue semaphore; multiplies
            # overlap the transfers.
            BL = O * C // NBLK
            for blk in range(NBLK):
                base = blk * 2 * BL
                nc.sync.dma_start(wmcat[:, base:base + 2 * BL],
                                  wmcat_d.ap()[:, base:base + 2 * BL])
                wm_eng = nc.vector if blk in WM_DVE_BLOCKS else nc.gpsimd
                wm_eng.tensor_mul(
                    wm16[:, blk * BL:(blk + 1) * BL],
                    wmcat[:, base:base + BL],
                    wmcat[:, base + BL:base + 2 * BL])

            # --- pooling: horizontal then vertical 3-tap box sums ---
            s1 = wpool.tile([C, 18 * 32], F32)
            s2 = wpool.tile([C, 18 * 32], F32)
            pt_raw = wpool.tile([C, L], F32)
            pt16 = wpool.tile([C, L], BF16)
            pt2 = wpool.tile([C, L], F32)
            x3 = xs[:].rearrange("c (h w) -> c h w", h=18)
            s1v = s1[:].rearrange("c (h w) -> c h w", h=18)
            s2v = s2[:].rearrange("c (h w) -> c h w", h=18)
            nc.vector.tensor_add(s1v, x3[:, :, 0:32], x3[:, :, 1:33])
            nc.vector.tensor_add(s2v, s1v, x3[:, :, 2:34])
            ptv = pt_raw[:].rearrange("c (h w) -> c h w", h=HS)
            pt2v = pt2[:].rearrange("c (h w) -> c h w", h=HS)
            nc.vector.tensor_add(pt2v, s2v[:, 0:16, :], s2v[:, 1:17, :])
            nc.vector.tensor_add(ptv, pt2v, s2v[:, 2:18, :])
            # p = boxsum/9; p enters the quadratic form twice -> 1/81 total
            nc.vector.tensor_scalar_mul(pt16[:], pt_raw[:], 1.0 / 9.0)

            y_ps = ypsum.tile([O, L], F32)
            for m in range(32):
                os3 = (3 * m, 3 * m + 1, 3 * m + 2)
                t3 = tpsum.tile([C, 3 * L], F32)
                for k, o in enumerate(os3):
                    nc.tensor.matmul(
                        t3[:, k * L:(k + 1) * L],
                        wm16[:, o * C:(o + 1) * C], pt16[:],
                        start=True, stop=True,
                    )
                z3 = zpool.tile([C, 3 * L], BF16, tag="z3")
                z3v = z3[:].rearrange("c (k l) -> c k l", k=3)
                pt3 = pt16[:].unsqueeze(1).broadcast_to((C, 3, L))
                if m in R1_TRIPLES:
                    nc.vector.tensor_mul(
                        z3v, t3[:].rearrange("c (k l) -> c k l", k=3), pt3)
                elif m in R3_TRIPLES:
                    tc3f = tcpool.tile([C, 3 * L], F32, tag="tc3f")
                    nc.scalar.activation(
                        tc3f[:], t3[:], mybir.ActivationFunctionType.Copy)
                    nc.gpsimd.tensor_mul(
                        z3v, tc3f[:].rearrange("c (k l) -> c k l", k=3), pt3)
                else:
                    tc3 = tcpool.tile([C, 3 * L], BF16, tag="tc3")
                    nc.scalar.activation(
                        tc3[:], t3[:], mybir.ActivationFunctionType.Copy)
                    nc.vector.tensor_mul(
                        z3v, tc3[:].rearrange("c (k l) -> c k l", k=3), pt3)
                for k, o in enumerate(os3):
                    # y row r(o) = 32*(o%3) + o//3 -> col group k, column m
                    nc.tensor.matmul(
                        y_ps[32 * k:32 * (k + 1), :],
                        zob[:, 31 - m:63 - m], z3[:, k * L:(k + 1) * L],
                        start=(m == 0), stop=(m == 31),
                        skip_group_check=True,
                    )


            y_sb = wpool.tile([O, L], F32)
            nc.vector.tensor_scalar_add(y_sb[:], y_ps[:], bias[:])
            # un-permute rows: SBUF row 32j+m holds output channel o=3m+j
            y_out = y_d.ap().rearrange("(m j) l -> j m l", j=3)
            nc.sync.dma_start(y_out, y_sb[:])
    return nc


_NC_CACHE = {}


def _get_nc():
    if "nc" not in _NC_CACHE:
        nc = bacc.Bacc("TRN2", target_bir_lowering=False, debug=False,
                       enable_asserts=False)
        _build_kernel(nc)
        nc.compile()
        _NC_CACHE["nc"] = nc
    return _NC_CACHE["nc"]


def _prep_shards(x, weight, mask, bias):
    xpad = np.pad(np.asarray(x, np.float32), ((0, 0), (0, 0), (1, 1), (1, 1)))
    wt = np.asarray(weight, np.float32).transpose(1, 0, 2).reshape(C, O * C)
    mt = np.asarray(mask, np.float32).transpose(1, 0, 2).reshape(C, O * C)
    BL = O * C // NBLK
    wmcat = np.empty((C, NBLK, 2, BL), np.float32)
    wmcat[:, :, 0, :] = wt.reshape(C, NBLK, BL)
    wmcat[:, :, 1, :] = mt.reshape(C, NBLK, BL)
    wmcat = np.ascontiguousarray(wmcat.reshape(C, 2 * O * C))
    # device keeps y rows permuted (row 32j+m = channel 3m+j); bias must
    # be laid out the same way
    b = np.asarray(bias, np.float32).reshape(32, 3).T.reshape(O, 1)
    b = np.ascontiguousarray(b)
    in_maps = []
    for core in range(N_CORES):
        n, half = core // 2, core % 2
        h0 = half * HS
        xs = np.ascontiguousarray(
            xpad[n, :, h0:h0 + 18, :].reshape(C, 18 * 34))
        in_maps.append({"xs": xs, "wmcat": wmcat, "bias": b})
    return in_maps


def run_sharded(x, weight, mask, bias, **run_kwargs):
    """Run on the 8 NeuronCores; returns (y_full, BassKernelResults)."""
    nc = _get_nc()
    in_maps = _prep_shards(x, weight, mask, bias)
    res = bass_utils.run_bass_kernel_spmd(
        nc, in_maps, core_ids=list(range(N_CORES)), **run_kwargs)
    n_img = np.asarray(x).shape[0]
    y = np.empty((n_img, O, 32, 32), dtype=np.float32)
    for core in range(N_CORES):
        n, half = core // 2, core % 2
        h0 = half * HS
        y[n, :, h0:h0 + HS, :] = res.results[core]["y"].reshape(O, HS, W)
    return y, res


def kernel(x, weight, mask, bias):
    y, _ = run_sharded(x, weight, mask, bias)
    return y


# revision 13
# speedup vs baseline: 1.1217x; 1.0652x over previous
"""Trainium2 Bass kernel: masked-bilinear channel-mixing Conv2d.

reference math (N=4, C=96, H=W=32, O=96, K=3, PAD=1):
    p = avgpool3x3(x, count_include_pad) -> [N, C, H, W] -> [N, L=1024, C]
    wm = weight * mask                              [O, C, C]
    y[n,l,o] = sum_{c,d} wm[o,c,d] p[n,l,c] p[n,l,d] + bias[o]

Sharding: data-parallel over the 4096 spatial locations -> 8 cores, each
takes half of one image (16 rows = 512 locations) and computes all 96
output channels. Weight/mask are replicated (host pre-transposes to
c-major, block-interleaved, so each block arrives in one contiguous DMA);
the avg-pool 1/9 scale and the weight*mask product run on device.

Per-core device pipeline (bf16 matmul operands, f32 PSUM accumulate):
  warmup: dummy matmul burst during the DMA/pool phase (the PE clock gate
          takes ~16us of sustained activity to lift 1.2GHz -> 2.4GHz)
  pooling (4 DVE adds + scale)        -> pt16 [96(c), 512(loc)] bf16
  wm16 = wt * mt                      -> [96(c), 9216(o,d)] bf16 (GPSIMD)
  o-loop, triples (3m, 3m+1, 3m+2):
    T_o  = matmul(lhsT=wm16[:, o], rhs=pt16)     # [96(d), 512] f32 PSUM
    z    = (T ⊙ pt16) batched per triple, routed to one of:
             direct DVE TT (PSUM src) | ACT copy + DVE bf16 2x TT |
             ACT copy + GPSIMD TT
    y[r(o)] = matmul(lhsT=onehot32, rhs=z_o)     # M=32; r(o)=32*(o%3)+o//3
             consecutive o's hit PE column-groups 0/1/2 -> 3x concurrent,
             and a triple shares one lhsT (same one-hot column m=o//3)
  y_sb = y + bias_perm; DMA out un-permutes rows via a reordered DRAM AP.
"""
import numpy as np

import concourse.bass as bass
import concourse.bacc as bacc
import concourse.mybir as mybir
from concourse import tile
from concourse import bass_utils

C = 96
O = 96
HS = 16           # rows per core shard
W = 32
L = HS * W        # 512 locations per core
N_CORES = 8
NBLK = 8          # weight/mask DMA + multiply blocks
WARMUP_MMS = 16
KEEPWARM_FROM = 99     # disabled
KEEPWARM_PER_TRIPLE = 0
F32 = mybir.dt.float32
BF16 = mybir.dt.bfloat16

# z-route per o-triple (m = o//3): R3 via ACT copy + GPSIMD TT,
# R1 direct DVE TT from PSUM, R2 (default) ACT copy + DVE bf16 2x TT.
R3_TRIPLES = set()
R1_TRIPLES = set()
WM_DVE_BLOCKS = {0, 1}   # rest go to GPSIMD


def _build_kernel(nc: bass.Bass):
    xs_d = nc.dram_tensor("xs", [C, 18 * 34], F32, kind="ExternalInput")
    wmcat_d = nc.dram_tensor("wmcat", [C, 2 * O * C], F32, kind="ExternalInput")
    b_d = nc.dram_tensor("bias", [O, 1], F32, kind="ExternalInput")
    y_d = nc.dram_tensor("y", [O, L], F32, kind="ExternalOutput")

    with tile.TileContext(nc) as tc:
        with (
            tc.tile_pool(name="const", bufs=1) as cpool,
            tc.tile_pool(name="work", bufs=1) as wpool,
            tc.tile_pool(name="tc3", bufs=3) as tcpool,
            tc.tile_pool(name="z", bufs=3) as zpool,
            tc.tile_pool(name="tpsum", bufs=2, space="PSUM") as tpsum,
            tc.tile_pool(name="ypsum", bufs=1, space="PSUM") as ypsum,
            tc.tile_pool(name="wpsum", bufs=1, space="PSUM") as wpsum,
        ):
            xs = cpool.tile([C, 18 * 34], F32)
            wmcat = cpool.tile([C, 2 * O * C], F32)
            wm16 = cpool.tile([C, O * C], BF16)
            bias = cpool.tile([O, 1], F32)
            # zob[:, 31] is ones, else zero; zob[:, 31-m : 63-m] is [96, 32]
            # with ones in column m -> as lhsT it scatters the partition-sum
            # of rhs into row (32*colgroup + m) of the output.
            zob = cpool.tile([C, 63], BF16)
            warm16 = cpool.tile([C, L], BF16)
            nc.sync.dma_start(xs[:], xs_d.ap())
            nc.sync.dma_start(bias[:], b_d.ap())
            nc.vector.memset(zob[:], 0.0)
            nc.vector.memset(zob[:, 31:32], 1.0)
            nc.vector.memset(warm16[:], 0.0)

            # PE warmup: garbage matmuls while DMA/pool/wm stages run
            wps = wpsum.tile([C, L], F32)
            for _ in range(WARMUP_MMS):
                nc.tensor.matmul(wps[:], warm16[:, 0:C], warm16[:],
                                 start=True, stop=True, skip_group_check=True)

            # weight/mask: host packs [blk] = [wt_blk | mt_blk] so each wm
            # block waits on exactly one DMA-queue semaphore; multiplies
            # overlap the transfers.
            BL = O * C // NBLK
            for blk in range(NBLK):
                base = blk * 2 * BL
                nc.sync.dma_start(wmcat[:, base:base + 2 * BL],
                                  wmcat_d.ap()[:, base:base + 2 * BL])
                wm_eng = nc.vector if blk in WM_DVE_BLOCKS else nc.gpsimd
                wm_eng.tensor_mul(
                    wm16[:, blk * BL:(blk + 1) * BL],
                    wmcat[:, base:base + BL],
                    wmcat[:, base + BL:base + 2 * BL])

            # --- pooling: horizontal then vertical 3-tap box sums ---
            s1 = wpool.tile([C, 18 * 32], F32)
            s2 = wpool.tile([C, 18 * 32], F32)
            pt_raw = wpool.tile([C, L], F32)
            pt16 = wpool.tile([C, L], BF16)
            pt2 = wpool.tile([C, L], F32)
            x3 = xs[:].rearrange("c (h w) -> c h w", h=18)
            s1v = s1[:].rearrange("c (h w) -> c h w", h=18)
            s2v = s2[:].rearrange("c (h w) -> c h w", h=18)
            nc.vector.tensor_add(s1v, x3[:, :, 0:32], x3[:, :, 1:33])
            nc.vector.tensor_add(s2v, s1v, x3[:, :, 2:34])
            ptv = pt_raw[:].rearrange("c (h w) -> c h w", h=HS)
            pt2v = pt2[:].rearrange("c (h w) -> c h w", h=HS)
            nc.vector.tensor_add(pt2v, s2v[:, 0:16, :], s2v[:, 1:17, :])
            nc.vector.tensor_add(ptv, pt2v, s2v[:, 2:18, :])
            # p = boxsum/9; p enters the quadratic form twice -> 1/81 total
            nc.vector.tensor_scalar_mul(pt16[:], pt_raw[:], 1.0 / 9.0)

            y_ps = ypsum.tile([O, L], F32)
            for m in range(32):
                os3 = (3 * m, 3 * m + 1, 3 * m + 2)
                t3 = tpsum.tile([C, 3 * L], F32)
                for k, o in enumerate(os3):
                    nc.tensor.matmul(
                        t3[:, k * L:(k + 1) * L],
                        wm16[:, o * C:(o + 1) * C], pt16[:],
                        start=True, stop=True,
                    )
                z3 = zpool.tile([C, 3 * L], BF16, tag="z3")
                z3v = z3[:].rearrange("c (k l) -> c k l", k=3)
                pt3 = pt16[:].unsqueeze(1).broadcast_to((C, 3, L))
                if m in R1_TRIPLES:
                    nc.vector.tensor_mul(
                        z3v, t3[:].rearrange("c (k l) -> c k l", k=3), pt3)
                elif m in R3_TRIPLES:
                    tc3f = tcpool.tile([C, 3 * L], F32, tag="tc3f")
                    nc.scalar.activation(
                        tc3f[:], t3[:], mybir.ActivationFunctionType.Copy)
                    nc.gpsimd.tensor_mul(
                        z3v, tc3f[:].rearrange("c (k l) -> c k l", k=3), pt3)
                else:
                    tc3 = tcpool.tile([C, 3 * L], BF16, tag="tc3")
                    nc.scalar.activation(
                        tc3[:], t3[:], mybir.ActivationFunctionType.Copy)
                    nc.vector.tensor_mul(
                        z3v, tc3[:].rearrange("c (k l) -> c k l", k=3), pt3)
                for k, o in enumerate(os3):
                    # y row r(o) = 32*(o%3) + o//3 -> col group k, column m
                    nc.tensor.matmul(
                        y_ps[32 * k:32 * (k + 1), :],
                        zob[:, 31 - m:63 - m], z3[:, k * L:(k + 1) * L],
                        start=(m == 0), stop=(m == 31),
                        skip_group_check=True,
                    )


            y_sb = wpool.tile([O, L], F32)
            nc.vector.tensor_scalar_add(y_sb[:], y_ps[:], bias[:])
            # un-permute rows: SBUF row 32j+m holds output channel o=3m+j
            y_out = y_d.ap().rearrange("(m j) l -> j m l", j=3)
            nc.sync.dma_start(y_out, y_sb[:])
    return nc


_NC_CACHE = {}


def _get_nc():
    if "nc" not in _NC_CACHE:
        nc = bacc.Bacc("TRN2", target_bir_lowering=False, debug=False,
                       enable_asserts=False)
        _build_kernel(nc)
        nc.compile()
        _NC_CACHE["nc"] = nc
    return _NC_CACHE["nc"]


def _prep_shards(x, weight, mask, bias):
    xpad = np.pad(np.asarray(x, np.float32), ((0, 0), (0, 0), (1, 1), (1, 1)))
    wt = np.asarray(weight, np.float32).transpose(1, 0, 2).reshape(C, O * C)
    mt = np.asarray(mask, np.float32).transpose(1, 0, 2).reshape(C, O * C)
    BL = O * C // NBLK
    wmcat = np.empty((C, NBLK, 2, BL), np.float32)
    wmcat[:, :, 0, :] = wt.reshape(C, NBLK, BL)
    wmcat[:, :, 1, :] = mt.reshape(C, NBLK, BL)
    wmcat = np.ascontiguousarray(wmcat.reshape(C, 2 * O * C))
    # device keeps y rows permuted (row 32j+m = channel 3m+j); bias must
    # be laid out the same way
    b = np.asarray(bias, np.float32).reshape(32, 3).T.reshape(O, 1)
    b = np.ascontiguousarray(b)
    in_maps = []
    for core in range(N_CORES):
        n, half = core // 2, core % 2
        h0 = half * HS
        xs = np.ascontiguousarray(
            xpad[n, :, h0:h0 + 18, :].reshape(C, 18 * 34))
        in_maps.append({"xs": xs, "wmcat": wmcat, "bias": b})
    return in_maps


def run_sharded(x, weight, mask, bias, **run_kwargs):
    """Run on the 8 NeuronCores; returns (y_full, BassKernelResults)."""
    nc = _get_nc()
    in_maps = _prep_shards(x, weight, mask, bias)
    res = bass_utils.run_bass_kernel_spmd(
        nc, in_maps, core_ids=list(range(N_CORES)), **run_kwargs)
    n_img = np.asarray(x).shape[0]
    y = np.empty((n_img, O, 32, 32), dtype=np.float32)
    for core in range(N_CORES):
        n, half = core // 2, core % 2
        h0 = half * HS
        y[n, :, h0:h0 + HS, :] = res.results[core]["y"].reshape(O, HS, W)
    return y, res


def kernel(x, weight, mask, bias):
    y, _ = run_sharded(x, weight, mask, bias)
    return y
